# revision 1
# baseline (speedup 1.0000x reference)
"""Trainium2 Bass kernel for causal GQA attention (B=1, T=4096, D=2048,
H=16, Hkv=4, Dh=128, RoPE) sharded over 8 NeuronCores.

Sharding: tensor-parallel over heads — each core owns 2 q-heads and the
kv head they share (core c: q-heads {2c, 2c+1}, kv head c//2). Each core
computes its q/k/v projections, RoPE, causal attention and its partial
o_proj contribution y_c = O_c @ Wo_c; the host sums the 8 partials.

On-device dataflow (everything bf16 into the PE, f32 accumulation):
  xT tiles [c,t] -> Q^T/K^T/V^T [d,t] -> RoPE (DVE) -> S^T = K^T.T@Q^T
  per (j-tile 128, q-tile 512) -> exp on ACT (PSUM->SBUF bf16, fused
  1/sqrt(dh) scale) -> causal mask on diagonal blocks (DVE mul with
  precomputed mask) -> O_aug[q,129] += P^T.T @ [V | 1] (PE, PSUM
  accumulation; col 128 gives the softmax denominator) -> reciprocal +
  per-partition scale on ACT -> PE transpose -> O^T -> y = O^T.T @ Wo.
"""

import sys

sys.path.insert(0, "/opt/trn_rl_repo")

import math
from contextlib import ExitStack

import ml_dtypes
import numpy as np

import concourse.bass as bass
import concourse.tile as tile
from concourse import bacc, mybir
from concourse.bass_utils import run_bass_kernel_spmd
from concourse.masks import make_identity

BF16 = mybir.dt.bfloat16
F32 = mybir.dt.float32
NPBF16 = ml_dtypes.bfloat16

B, T, D = 1, 4096, 2048
H, HKV, DH = 16, 4, 128
GROUP = H // HKV
ROPE_BASE = 10000.0
N_CORES = 8
HL = H // N_CORES  # q-heads per core
KC = D // 128      # contraction tiles for projections
NQ = T // 512      # 512-wide q tiles
NJ = T // 128      # 128-wide kv tiles
NT = T // 128      # 128-row output tiles
NM = D // 512      # 512-wide output column tiles
SCALE = 1.0 / math.sqrt(DH)

Exp = mybir.ActivationFunctionType.Exp
Copy = mybir.ActivationFunctionType.Copy


def _build(nc):
    xp = nc.dram_tensor("xp", [NQ, 128, KC, 512], BF16, kind="ExternalInput").ap()
    wqkv = nc.dram_tensor("wqkv", [128, KC, 4, 128], BF16, kind="ExternalInput").ap()
    wo = nc.dram_tensor("wo", [128, HL, D], BF16, kind="ExternalInput").ap()
    cos2 = nc.dram_tensor("cos2", [128, T], BF16, kind="ExternalInput").ap()
    sinsig = nc.dram_tensor("sinsig", [128, T], BF16, kind="ExternalInput").ap()
    perm = nc.dram_tensor("perm", [128, 128], BF16, kind="ExternalInput").ap()
    y = nc.dram_tensor("y", [T, D], BF16, kind="ExternalOutput").ap()

    with tile.TileContext(nc) as tc, ExitStack() as ctx:
        const = ctx.enter_context(tc.tile_pool(name="const", bufs=1))
        xpool = ctx.enter_context(tc.tile_pool(name="xp", bufs=2))
        psum = ctx.enter_context(tc.tile_pool(name="ps", bufs=4, space="PSUM"))
        opsum = ctx.enter_context(tc.tile_pool(name="ops", bufs=2, space="PSUM"))
        auxp = ctx.enter_context(tc.tile_pool(name="aux", bufs=2, space="PSUM"))
        ppool = ctx.enter_context(tc.tile_pool(name="pt", bufs=6))
        swpool = ctx.enter_context(tc.tile_pool(name="sw", bufs=6))
        spool = ctx.enter_context(tc.tile_pool(name="sm", bufs=4))
        bcpool = ctx.enter_context(tc.tile_pool(name="bc", bufs=2))
        yrow = ctx.enter_context(tc.tile_pool(name="yr", bufs=2))

        wqkv_sb = const.tile([128, KC, 4, 128], BF16, tag="wqkv")
        wo_sb = const.tile([128, HL, D], BF16, tag="wo")
        cos_sb = const.tile([128, T], BF16, tag="cos")
        sin_sb = const.tile([128, T], BF16, tag="sin")
        perm_sb = const.tile([128, 128], BF16, tag="perm")
        ident = const.tile([128, 128], BF16, tag="ident")
        qkvT = const.tile([128, 4, T], BF16, tag="qkvT")   # Q0,Q1,K,V as [d,t]; RoPE in place
        vnat = const.tile([128, NJ, 128], BF16, tag="vnat")  # V natural [j, d]
        ones_sb = const.tile([128, 128], BF16, tag="ones")
        oT = const.tile([128, HL, T], BF16, tag="oT")

        make_identity(nc, ident[:])
        nc.vector.memset(ones_sb[:], 1.0)

        def emit_oproj(oq):
            # partial o_proj y = O^T.T @ Wo for q-tile oq's 4 row blocks;
            # PSUM->SBUF casts alternate DVE/ACT to avoid a single-engine drain.
            for tsub in range(4):
                ti = oq * 4 + tsub
                yr = yrow.tile([128, D], BF16, tag="yr", name=f"yr{ti}")
                for mi in range(NM):
                    yp = psum.tile([128, 512], F32, tag="ps", name=f"yp{ti}_{mi}")
                    for h2 in range(HL):
                        nc.tensor.matmul(
                            yp[:],
                            lhsT=oT[:, h2, bass.ts(ti, 128)],
                            rhs=wo_sb[:, h2, bass.ts(mi, 512)],
                            start=(h2 == 0),
                            stop=(h2 == HL - 1),
                        )
                    if mi % 2 == 0:
                        nc.vector.tensor_copy(yr[:, bass.ts(mi, 512)], yp[:])
                    else:
                        nc.scalar.copy(yr[:, bass.ts(mi, 512)], yp[:])
                nc.gpsimd.dma_start(y[bass.ts(ti, 128), :], yr[:])

        kT = qkvT[:, 2, :]
        for n in range(NQ):
            ns = bass.ts(n, 512)
            # x tile for this 512-token window (split so transfers start early)
            xt = xpool.tile([128, KC, 512], BF16, tag="xt")
            for kq in range(4):
                nc.gpsimd.dma_start(
                    xt[:, bass.ts(kq, 4), :], xp[n, :, bass.ts(kq, 4), :]
                )
                if n == 0:  # interleave weight chunks with the first x tile
                    nc.gpsimd.dma_start(
                        wqkv_sb[:, bass.ts(kq, 4)], wqkv[:, bass.ts(kq, 4)]
                    )
            if n == 0:
                nc.sync.dma_start(perm_sb[:], perm[:])
                nc.sync.dma_start(cos_sb[:], cos2[:])
                nc.sync.dma_start(sin_sb[:], sinsig[:])
                nc.sync.dma_start(wo_sb[:], wo[:])

            # fused q/k/v projection for this window, outputs transposed [d, t]
            for m in range(4):
                ps = psum.tile([128, 512], F32, tag="ps", name=f"prj{n}_{m}")
                for k in range(KC):
                    nc.tensor.matmul(
                        ps[:],
                        lhsT=wqkv_sb[:, k, m, :],
                        rhs=xt[:, k, :],
                        start=(k == 0),
                        stop=(k == KC - 1),
                    )
                nc.scalar.copy(qkvT[:, m, ns], ps[:])

            # RoPE for this window, q heads first: the attention q-tile
            # needs them immediately, while k of THIS window is only read at
            # the tail of the kv loop (jt >= 4n).
            # rotate_half partition swap runs as a permutation matmul on the
            # PE (elementwise engines cannot shift partitions), then
            # src = src*cos + swap(src)*[-sin; sin] in place on DVE.
            for i in (0, 1, 2):
                src = qkvT[:, i, ns]
                sw_ps = psum.tile([128, 512], F32, tag="ps", name=f"swp{n}_{i}")
                nc.tensor.matmul(
                    sw_ps[:], lhsT=perm_sb[:], rhs=src, start=True, stop=True
                )
                swp = swpool.tile([128, 512], BF16, tag="sw", name=f"sw{n}_{i}")
                nc.scalar.copy(swp[:], sw_ps[:])
                nc.vector.tensor_mul(src, src, cos_sb[:, ns])
                nc.vector.tensor_mul(swp[:], swp[:], sin_sb[:, ns])
                nc.vector.tensor_add(src, src, swp[:])

            # V^T -> V natural [j, d] for this window's 4 kv tiles (PE transpose)
            for jt in range(4 * n, 4 * n + 4):
                tp = auxp.tile([128, 128], BF16, tag="aux", name=f"vtp{jt}")
                nc.tensor.transpose(tp[:], qkvT[:, 3, bass.ts(jt, 128)], ident[:])
                nc.vector.tensor_copy(vnat[:, jt, :], tp[:])

            # causal attention for q-tile qi=n, both heads interleaved per kv
            # tile (they share K/V): S^T = K^T.T @ Q^T -> exp on ACT
            # (PSUM->SBUF bf16, fused 1/sqrt(dh)) -> causal mask on diagonal
            # blocks (gpsimd affine_select in place) -> PE accumulations
            # O^T[d,q] += V.T @ P^T and l += ones.T @ P^T (every row of lb
            # is the same column sum) -> O^T normalized by 1/l on DVE.
            qi = n
            njt = 4 * (qi + 1)
            ot = [
                opsum.tile([128, 512], F32, tag="oacc", name=f"oacc{qi}_{h}")
                for h in range(HL)
            ]
            lb = [
                auxp.tile([128, 512], F32, tag="aux", name=f"lacc{qi}_{h}")
                for h in range(HL)
            ]
            for jt in range(njt):
                kd = jt - 4 * qi
                pts = []
                for h in range(HL):
                    sps = psum.tile(
                        [128, 512], F32, tag="ps", name=f"sps{qi}_{jt}_{h}"
                    )
                    nc.tensor.matmul(
                        sps[:],
                        lhsT=kT[:, bass.ts(jt, 128)],
                        rhs=qkvT[:, h, bass.ts(qi, 512)],
                        start=True,
                        stop=True,
                    )
                    pt = ppool.tile([128, 512], BF16, tag="pt", name=f"pt{jt}_{h}")
                    nc.scalar.activation(pt[:], sps[:], Exp, scale=SCALE)
                    if kd >= 0:  # block straddles the diagonal
                        nc.gpsimd.affine_select(
                            out=pt[:],
                            in_=pt[:],
                            compare_op=mybir.AluOpType.is_ge,
                            fill=0.0,
                            base=-kd * 128,
                            channel_multiplier=-1,
                            pattern=[[1, 512]],
                        )
                    pts.append(pt)
                for h in range(HL):
                    nc.tensor.matmul(
                        ot[h][:],
                        lhsT=vnat[:, jt, :],
                        rhs=pts[h][:],
                        start=(jt == 0),
                        stop=(jt == njt - 1),
                    )
                for h in range(HL):
                    nc.tensor.matmul(
                        lb[h][:],
                        lhsT=ones_sb[:],
                        rhs=pts[h][:],
                        start=(jt == 0),
                        stop=(jt == njt - 1),
                    )
            for h in range(HL):
                bc = bcpool.tile([128, 512], F32, tag="bc", name=f"bc{qi}_{h}")
                nc.vector.reciprocal_approx_fast(bc[:], lb[h][:])
                nc.vector.tensor_mul(oT[:, h, bass.ts(qi, 512)], ot[h][:], bc[:])

            # o_proj for the previous q-tile (delayed so the PE has ready
            # work while this q-tile's normalize drains on DVE)
            if qi > 0:
                emit_oproj(qi - 1)
        emit_oproj(NQ - 1)


_CACHE = {}


def _get_program():
    if "nc" not in _CACHE:
        nc = bacc.Bacc(
            "TRN2", target_bir_lowering=False, debug=False, num_devices=N_CORES
        )
        _build(nc)
        nc.compile()
        _CACHE["nc"] = nc
    return _CACHE["nc"]


def _rope_tables():
    inv_freq = 1.0 / (ROPE_BASE ** (np.arange(64, dtype=np.float64) / 64))
    ang = np.arange(T, dtype=np.float64)[:, None] * inv_freq[None, :]  # [T, 64]
    cos = np.cos(ang).T  # [64, T]
    sin = np.sin(ang).T
    cos2 = np.concatenate([cos, cos], axis=0).astype(NPBF16)
    sinsig = np.concatenate([-sin, sin], axis=0).astype(NPBF16)
    return cos2, sinsig


def kernel(x, Wq, Wk, Wv, Wo):
    x = np.asarray(x, dtype=np.float32)
    Wq = np.asarray(Wq, dtype=np.float32)
    Wk = np.asarray(Wk, dtype=np.float32)
    Wv = np.asarray(Wv, dtype=np.float32)
    Wo = np.asarray(Wo, dtype=np.float32)

    # x[t, c] -> xp[n, p, k, j] = x[n*512+j, k*128+p]; contiguous per partition.
    xp = np.ascontiguousarray(
        x.reshape(T, D).reshape(NQ, 512, KC, 128).transpose(0, 3, 2, 1)
    ).astype(NPBF16)
    cos2, sinsig = _rope_tables()
    d_idx = np.arange(128)
    permm = (d_idx[:, None] == (d_idx[None, :] + 64) % 128).astype(NPBF16)

    in_maps = []
    for c in range(N_CORES):
        h0, h1 = 2 * c, 2 * c + 1
        kv = c // 2
        wqkv_c = np.concatenate(
            [
                Wq[:, h0 * DH:(h0 + 1) * DH],
                Wq[:, h1 * DH:(h1 + 1) * DH],
                Wk[:, kv * DH:(kv + 1) * DH],
                Wv[:, kv * DH:(kv + 1) * DH],
            ],
            axis=1,
        )  # [D, 512]
        wqkv_pre = np.ascontiguousarray(
            wqkv_c.reshape(KC, 128, 4, 128).transpose(1, 0, 2, 3)
        ).astype(NPBF16)
        wo_pre = np.ascontiguousarray(
            np.stack(
                [Wo[h0 * DH:(h0 + 1) * DH, :], Wo[h1 * DH:(h1 + 1) * DH, :]], axis=0
            ).transpose(1, 0, 2)
        ).astype(NPBF16)
        in_maps.append(
            {
                "xp": xp,
                "wqkv": wqkv_pre,
                "wo": wo_pre,
                "cos2": cos2,
                "sinsig": sinsig,
                "perm": permm,
            }
        )

    nc = _get_program()
    res = run_bass_kernel_spmd(nc, in_maps, list(range(N_CORES)))
    out = np.zeros((T, D), dtype=np.float32)
    for c in range(N_CORES):
        out += res.results[c]["y"].astype(np.float32)
    return out.reshape(B, T, D)

